# revision 23
# baseline (speedup 1.0000x reference)
"""Trainium2 Bass kernel for nn_LIANNmodel (moe_routing).

Model (B=512, S=32 experts, H=512, D=256, C=128):
    z[b,s,h]   = sum_d x[b,d] * W0[s,h,d] + b0[s,h]
    xTop       = max_s z ; xIndex = argmax_s z
    topK[h,d]  = mean_b( W0[xIndex[b,h], h, d] + LR*sign(x[b,d]) )
    logits     = relu(xTop) @ W1.T + b1 ; logp = log_softmax(logits)
    loss, acc  from logp vs labels y

Sharding: expert-sharded over S across 8 cores (4 experts/core).  Each core
computes z^T (h on partitions, b on free) for its experts over the full
batch, reduces a local max, and an AllReduce(max) of [H,B] produces the
global xTop^T on every core.  Routing counts are recovered locally with a
fused is_equal+reduce against xTop^T (no all-to-all needed), and the
gather-mean  sum_b W0[xIndex[b,h],h,:]  is computed on the tensor engine as
sum_s  W0[s]-native-block^T @ diag(count[s,.])  accumulated in PSUM.  The
classifier head is computed redundantly on every core (xTop^T is already
the needed lhsT).  The host only shards inputs, sums the 8 partial topK
outputs, and computes the two scalar reductions (loss/accuracy) from the
device-produced logp.

Layout rules learned from hardware: only contiguous DMA (strided/transposed
DRAM reads run at ~5 GB/s vs 165 GB/s); all transposes happen on the host
(x.T, W1.T, b0.T - free) or on the PE via identity matmuls (W0 blocks).
The z^T layout makes b0 a per-partition column folded into the PSUM->SBUF
copy on the scalar engine, and makes counts per-partition columns exactly
as the diag build needs - no bias matmuls, no count/xTop transposes.
"""

import sys
import os
import numpy as np

for _p in ("/opt/trn_rl_repo", "/root/.axon_site/_ro/trn_rl_repo"):
    if os.path.isdir(_p) and _p not in sys.path:
        sys.path.append(_p)

import concourse.bacc as bacc
from concourse.tile import add_dep_helper
import concourse.mybir as mybir
import concourse.tile as tile
from concourse import bass_utils

F32 = mybir.dt.float32
AF = mybir.ActivationFunctionType
OP = mybir.AluOpType

B, S, H, D, C = 512, 32, 512, 256, 128
N_CORES = 8
SL = S // N_CORES          # experts per core (shard length)
NBT = B // 128             # batch tiles of 128
NHC = H // 128             # h chunks of 128
NDC = D // 128             # d chunks of 128
LIANN_LR = 0.001

_CACHED_NC = None
# tensor_tensor_reduce crashes at runtime on HW (sim-only ok)
K_TTR = os.environ.get("K_TTR", "0") == "1"
K_DMA3D = os.environ.get("K_DMA3D", "1") == "1"
# fp32r: reduced-precision multiply, 4x matmul stream rate at N>=256
K_F32R = os.environ.get("K_F32R", "0") == "1"


def build_nc():
    nc = bacc.Bacc("TRN2", target_bir_lowering=False, debug=False,
                   num_devices=N_CORES)

    # ---- kernel I/O (per-core shapes; contents differ per core) ----
    w0n = nc.dram_tensor("w0n", [SL, H, D], F32, kind="ExternalInput")
    w0t = nc.dram_tensor("w0t", [SL, D, H], F32, kind="ExternalInput")
    b0c = nc.dram_tensor("b0c", [H, SL], F32, kind="ExternalInput")  # b0.T
    xt = nc.dram_tensor("xt", [D, B], F32, kind="ExternalInput")     # x.T
    w1t = nc.dram_tensor("w1t", [H, C], F32, kind="ExternalInput")   # W1.T
    b1 = nc.dram_tensor("b1", [1, C], F32, kind="ExternalInput")
    ident = nc.dram_tensor("ident", [128, 128], F32, kind="ExternalInput")

    logp_out = nc.dram_tensor("logp", [B, C], F32, kind="ExternalOutput")
    # transposed partial: sum_{s in shard} count[s,h]*W0[s,h,d]  (+ upd share)
    topkt_out = nc.dram_tensor("topkt", [D, H], F32, kind="ExternalOutput")

    with tile.TileContext(nc) as tc:
        with (
            tc.tile_pool(name="const", bufs=1) as constp,
            tc.tile_pool(name="big", bufs=1) as bigp,
            tc.tile_pool(name="work", bufs=3) as workp,
            tc.tile_pool(name="pz", bufs=6, space="PSUM") as pzp,
            tc.tile_pool(name="psm", bufs=1, space="PSUM") as psmp,
            tc.tile_pool(name="dram", bufs=1, space="DRAM") as dramp,
        ):
            # ---------- constants / small inputs ----------
            identt = constp.tile([128, 128], F32, tag="identt")
            nc.sync.dma_start(identt[:], ident[:])
            ones_row = constp.tile([1, 128], F32, tag="ones_row")
            nc.vector.memset(ones_row[:], 1.0)
            b1t = constp.tile([1, C], F32, tag="b1t")
            nc.sync.dma_start(b1t[:], b1[:])
            # einsum operands first: w0t shard (lhsT) + xT (rhs) gate
            # the critical path; w0n (weighted-sum, post-AR) goes last.
            # w0tt free layout: (s, dc) chunk of [128_d, 512_h]
            w0tt = bigp.tile([128, SL * NDC * H], F32, tag="w0tt")
            xtt = constp.tile([128, NDC * B], F32, tag="xtt")

            def load_w0t(s):
                return nc.sync.dma_start(
                    w0tt[:, s * NDC * H:(s + 1) * NDC * H].rearrange(
                        "p (c h) -> p c h", c=NDC),
                    w0t[s].rearrange("(c p) h -> p c h", p=128))

            # serialize the front loads: s0+xT get full DMA bandwidth so the
            # first matmul starts ~6us in; later chunks chase the einsum.
            i_prev = load_w0t(0)
            i_xtt = nc.sync.dma_start(
                xtt[:].rearrange("p (c b) -> p c b", c=NDC),
                xt[:].rearrange("(c p) b -> p c b", p=128))
            # b0 columns: [128_h, (hc, s)]
            b0cc = constp.tile([128, NHC * SL], F32, tag="b0cc")
            i_b0 = nc.sync.dma_start(
                b0cc[:].rearrange("p (c s) -> p c s", c=NHC),
                b0c[:].rearrange("(c p) s -> p c s", p=128))
            add_dep_helper(i_b0.ins, i_prev.ins, reason="front DMA ordering")
            for s in range(1, SL):
                i_s = load_w0t(s)
                add_dep_helper(i_s.ins, i_prev.ins,
                               reason="front DMA ordering")
                i_prev = i_s
            w1tt = constp.tile([128, NHC * C], F32, tag="w1tt")
            i_w1 = nc.sync.dma_start(
                w1tt[:].rearrange("p (c k) -> p c k", c=NHC),
                w1t[:].rearrange("(c p) k -> p c k", p=128))
            add_dep_helper(i_w1.ins, i_prev.ins, reason="front DMA ordering")
            w0nt = bigp.tile([128, SL * NHC * D], F32, tag="w0nt")

            zts = {}
            xtl = []
            cc_in = dramp.tile([H, B], F32, tag="cc_in")
            cc_out = dramp.tile([H, B], F32, tag="cc_out", addr_space="Shared")
            for hc in range(NHC):
                mt = bigp.tile([128, B], F32, tag=f"xtl{hc}")
                xtl.append(mt)
            for s in range(SL):
                for hc in range(NHC):
                    pz = pzp.tile([128, B], F32, tag="pz")
                    for dc in range(NDC):
                        lhs = w0tt[:, (s * NDC + dc) * H + hc * 128:
                                   (s * NDC + dc) * H + (hc + 1) * 128]
                        rhs = xtt[:, dc * B:(dc + 1) * B]
                        if K_F32R:
                            lhs = lhs.bitcast(mybir.dt.float32r)
                            rhs = rhs.bitcast(mybir.dt.float32r)
                        nc.tensor.matmul(
                            pz[:], lhs, rhs,
                            start=(dc == 0), stop=(dc == NDC - 1))
                    zt = bigp.tile([128, B], F32, tag=f"z{s}_{hc}")
                    zts[(s, hc)] = zt
                    nc.scalar.activation(
                        zt[:], pz[:], AF.Identity,
                        bias=b0cc[:, hc * SL + s:hc * SL + s + 1])
                    # incremental running max per hc
                    if s == 1:
                        nc.vector.tensor_tensor(
                            out=xtl[hc][:], in0=zts[(0, hc)][:],
                            in1=zt[:], op=OP.max)
                    elif s > 1:
                        nc.vector.tensor_tensor(
                            out=xtl[hc][:], in0=xtl[hc][:],
                            in1=zt[:], op=OP.max)
                    if s == SL - 1:
                        nc.sync.dma_start(
                            cc_in[hc * 128:(hc + 1) * 128, :], xtl[hc][:])
            nc.gpsimd.collective_compute(
                "AllReduce", OP.max,
                replica_groups=[list(range(N_CORES))],
                ins=[cc_in[:].opt()], outs=[cc_out[:].opt()])
            # w0n (weighted-sum operand) loads during the AR dead time
            for s in range(SL):
                nc.sync.dma_start(
                    w0nt[:, s * NHC * D:(s + 1) * NHC * D].rearrange(
                        "p (c d) -> p c d", c=NHC),
                    w0n[s].rearrange("(c p) d -> p c d", p=128))
            gxt = []
            xrel = []
            for hc in range(NHC):
                gt = bigp.tile([128, B], F32, tag=f"g{hc}")
                gxt.append(gt)
                nc.sync.dma_start(gt[:], cc_out[hc * 128:(hc + 1) * 128, :])
                # relu early so the logits/softmax chain starts with eq
                xr = bigp.tile([128, B], F32, tag=f"xr{hc}")
                xrel.append(xr)
                nc.vector.tensor_scalar(
                    out=xr[:], in0=gt[:], scalar1=0.0, scalar2=None,
                    op0=OP.max)

            # ---------- upd share from sign(x) (independent of AR) -------
            # sum_b sign(x) = #pos - #neg, via tensor_scalar accumulators
            updcol = workp.tile([128, NDC], F32, tag="updcol", bufs=1)
            for dc in range(NDC):
                sx = workp.tile([128, B], F32, tag="sx", bufs=2)
                pos = workp.tile([128, 1], F32, tag="pos", bufs=2)
                neg = workp.tile([128, 1], F32, tag="neg", bufs=2)
                nc.vector.tensor_scalar(
                    out=sx[:], in0=xtt[:, dc * B:(dc + 1) * B],
                    scalar1=0.0, scalar2=0.0, op0=OP.is_gt, op1=OP.add,
                    accum_out=pos[:])
                nc.vector.tensor_scalar(
                    out=sx[:], in0=xtt[:, dc * B:(dc + 1) * B],
                    scalar1=0.0, scalar2=0.0, op0=OP.is_lt, op1=OP.add,
                    accum_out=neg[:])
                nc.vector.tensor_scalar(
                    out=updcol[:, dc:dc + 1], in0=pos[:],
                    scalar1=neg[:], scalar2=LIANN_LR / N_CORES,
                    op0=OP.subtract, op1=OP.mult)

            # ---------- fused eq+count, diag, weighted sum ----------
            countsT = workp.tile([128, NHC * SL], F32, tag="countsT", bufs=1)
            ptop = []
            for dc in range(NDC):
                pt = psmp.tile([128, H], F32, tag=f"ptop{dc}", bufs=1)
                ptop.append(pt)
            for hc in range(NHC):
                dgs = []
                for s in range(SL):
                    eqs = workp.tile([128, B], F32, tag="eqs", bufs=3)
                    if K_TTR:
                        nc.vector.tensor_tensor_reduce(
                            out=eqs[:], in0=zts[(s, hc)][:], in1=gxt[hc][:],
                            scale=1.0, scalar=0.0,
                            op0=OP.is_equal, op1=OP.add,
                            accum_out=countsT[:, hc * SL + s:hc * SL + s + 1])
                    else:
                        nc.vector.tensor_tensor(
                            out=eqs[:], in0=zts[(s, hc)][:], in1=gxt[hc][:],
                            op=OP.is_equal)
                        eq2 = workp.tile([128, B], F32, tag="eq2", bufs=3)
                        nc.scalar.activation(
                            eq2[:], eqs[:], AF.Identity,
                            accum_out=countsT[:, hc * SL + s:hc * SL + s + 1])
                    dg = workp.tile([128, 128], F32, tag=f"dg{s}", bufs=2)
                    dgs.append(dg)
                    nc.vector.tensor_scalar(
                        out=dg[:], in0=identt[:],
                        scalar1=countsT[:, hc * SL + s:hc * SL + s + 1],
                        scalar2=None, op0=OP.mult)
                for dc in range(NDC):
                    for s in range(SL):
                        nc.tensor.matmul(
                            ptop[dc][:, hc * 128:(hc + 1) * 128],
                            w0nt[:, (s * NHC + hc) * D + dc * 128:
                                 (s * NHC + hc) * D + (dc + 1) * 128],
                            dgs[s][:],
                            start=(s == 0), stop=(s == SL - 1))

            for dc in range(NDC):
                tk = workp.tile([128, H], F32, tag="tk", bufs=2)
                nc.vector.tensor_copy(tk[:], ptop[dc][:])
                nc.vector.tensor_scalar(
                    out=tk[:], in0=tk[:],
                    scalar1=updcol[:, dc:dc + 1], scalar2=None,
                    op0=OP.add)
                nc.sync.dma_start(topkt_out[dc * 128:(dc + 1) * 128, :], tk[:])

            # ---------- head: relu(xTop^T) @ W1^T + b1, log_softmax ------
            for bt in range(NBT):
                pl = pzp.tile([128, C], F32, tag="pz")
                for hc in range(NHC):
                    nc.tensor.matmul(
                        pl[:],
                        xrel[hc][:, bt * 128:(bt + 1) * 128],
                        w1tt[:, hc * C:(hc + 1) * C],
                        start=(hc == 0), stop=False)
                nc.tensor.matmul(
                    pl[:], ones_row[:], b1t[:], start=False, stop=True)
                # log-softmax over free dim (C)
                mx = workp.tile([128, 1], F32, tag="mx")
                nc.vector.reduce_max(mx[:], pl[:], axis=mybir.AxisListType.X)
                sh = workp.tile([128, C], F32, tag="sh")
                nc.vector.tensor_scalar(
                    out=sh[:], in0=pl[:], scalar1=mx[:], scalar2=None,
                    op0=OP.subtract)
                ex = workp.tile([128, C], F32, tag="ex")
                se = workp.tile([128, 1], F32, tag="se")
                nc.scalar.activation(ex[:], sh[:], AF.Exp, accum_out=se[:])
                ls = workp.tile([128, 1], F32, tag="ls")
                nc.scalar.activation(ls[:], se[:], AF.Ln)
                lp = workp.tile([128, C], F32, tag="lp")
                nc.vector.tensor_scalar(
                    out=lp[:], in0=sh[:], scalar1=ls[:], scalar2=None,
                    op0=OP.subtract)
                nc.sync.dma_start(logp_out[bt * 128:(bt + 1) * 128, :], lp[:])

    nc.finalize()
    return nc


def get_nc():
    global _CACHED_NC
    if _CACHED_NC is None:
        _CACHED_NC = build_nc()
    return _CACHED_NC


def make_in_maps(x, W0, b0, W1, b1):
    x = np.asarray(x, np.float32)
    W0 = np.asarray(W0, np.float32)
    b0 = np.asarray(b0, np.float32)
    W1 = np.asarray(W1, np.float32)
    b1 = np.asarray(b1, np.float32)

    xt = np.ascontiguousarray(x.T)                    # [D, B]
    w1t = np.ascontiguousarray(W1.T)                  # [H, C]
    b1r = np.ascontiguousarray(b1.reshape(1, C))
    ident = np.eye(128, dtype=np.float32)

    in_maps = []
    for c in range(N_CORES):
        sl = slice(c * SL, (c + 1) * SL)
        in_maps.append({
            "w0n": np.ascontiguousarray(W0[sl]),
            "w0t": np.ascontiguousarray(W0[sl].transpose(0, 2, 1)),
            "b0c": np.ascontiguousarray(b0[sl].T),    # [H, SL]
            "xt": xt,
            "w1t": w1t,
            "b1": b1r,
            "ident": ident,
        })
    return in_maps


def postprocess(results, y):
    logp = results[0]["logp"]
    topkt = sum(r["topkt"] for r in results)
    topk = np.ascontiguousarray(topkt.T) / np.float32(B)

    yi = np.asarray(y).astype(np.int64)
    loss = np.float32(-np.mean(logp[np.arange(B), yi]))
    acc = np.float32(np.mean((np.argmax(logp, axis=-1) == yi)))
    return loss, acc, logp, topk


def kernel(x, y, W0, b0, W1, b1):
    nc = get_nc()
    in_maps = make_in_maps(x, W0, b0, W1, b1)
    res = bass_utils.run_bass_kernel_spmd(
        nc, in_maps, core_ids=list(range(N_CORES)))
    return postprocess(res.results, y)


# revision 25
# speedup vs baseline: 1.1096x; 1.1096x over previous
"""Trainium2 Bass kernel for nn_LIANNmodel (moe_routing).

Model (B=512, S=32 experts, H=512, D=256, C=128):
    z[b,s,h]   = sum_d x[b,d] * W0[s,h,d] + b0[s,h]
    xTop       = max_s z ; xIndex = argmax_s z
    topK[h,d]  = mean_b( W0[xIndex[b,h], h, d] + LR*sign(x[b,d]) )
    logits     = relu(xTop) @ W1.T + b1 ; logp = log_softmax(logits)
    loss, acc  from logp vs labels y

Sharding: expert-sharded over S across 8 cores (4 experts/core).  Each core
computes z^T (h on partitions, b on free) for its experts over the full
batch, reduces a local max, and an AllReduce(max) of [H,B] produces the
global xTop^T on every core.  Routing counts are recovered locally with a
fused is_equal+reduce against xTop^T (no all-to-all needed), and the
gather-mean  sum_b W0[xIndex[b,h],h,:]  is computed on the tensor engine as
sum_s  W0[s]-native-block^T @ diag(count[s,.])  accumulated in PSUM.  The
classifier head is computed redundantly on every core (xTop^T is already
the needed lhsT).  The host only shards inputs, sums the 8 partial topK
outputs, and computes the two scalar reductions (loss/accuracy) from the
device-produced logp.

Layout rules learned from hardware: only contiguous DMA (strided/transposed
DRAM reads run at ~5 GB/s vs 165 GB/s); all transposes happen on the host
(x.T, W1.T, b0.T - free) or on the PE via identity matmuls (W0 blocks).
The z^T layout makes b0 a per-partition column folded into the PSUM->SBUF
copy on the scalar engine, and makes counts per-partition columns exactly
as the diag build needs - no bias matmuls, no count/xTop transposes.
"""

import sys
import os
import numpy as np

for _p in ("/opt/trn_rl_repo", "/root/.axon_site/_ro/trn_rl_repo"):
    if os.path.isdir(_p) and _p not in sys.path:
        sys.path.append(_p)

import concourse.bacc as bacc
from concourse.tile import add_dep_helper
import concourse.mybir as mybir
import concourse.tile as tile
from concourse import bass_utils

F32 = mybir.dt.float32
AF = mybir.ActivationFunctionType
OP = mybir.AluOpType

B, S, H, D, C = 512, 32, 512, 256, 128
N_CORES = 8
SL = S // N_CORES          # experts per core (shard length)
NBT = B // 128             # batch tiles of 128
NHC = H // 128             # h chunks of 128
NDC = D // 128             # d chunks of 128
LIANN_LR = 0.001

_CACHED_NC = None
# tensor_tensor_reduce crashes at runtime on HW (sim-only ok)
K_TTR = os.environ.get("K_TTR", "0") == "1"
K_DMA3D = os.environ.get("K_DMA3D", "1") == "1"
# fp32r: reduced-precision multiply, 4x matmul stream rate at N>=256
K_F32R = os.environ.get("K_F32R", "0") == "1"


def build_nc():
    nc = bacc.Bacc("TRN2", target_bir_lowering=False, debug=False,
                   num_devices=N_CORES)

    # ---- kernel I/O (per-core shapes; contents differ per core) ----
    w0n = nc.dram_tensor("w0n", [SL, H, D], F32, kind="ExternalInput")
    w0t = nc.dram_tensor("w0t", [SL, D, H], F32, kind="ExternalInput")
    b0c = nc.dram_tensor("b0c", [H, SL], F32, kind="ExternalInput")  # b0.T
    xt = nc.dram_tensor("xt", [D, B], F32, kind="ExternalInput")     # x.T
    w1t = nc.dram_tensor("w1t", [H, C], F32, kind="ExternalInput")   # W1.T
    b1 = nc.dram_tensor("b1", [1, C], F32, kind="ExternalInput")
    ident = nc.dram_tensor("ident", [128, 128], F32, kind="ExternalInput")

    logp_out = nc.dram_tensor("logp", [B, C], F32, kind="ExternalOutput")
    # transposed partial: sum_{s in shard} count[s,h]*W0[s,h,d]  (+ upd share)
    topkt_out = nc.dram_tensor("topkt", [D, H], F32, kind="ExternalOutput")

    with tile.TileContext(nc) as tc:
        with (
            tc.tile_pool(name="const", bufs=1) as constp,
            tc.tile_pool(name="big", bufs=1) as bigp,
            tc.tile_pool(name="work", bufs=3) as workp,
            tc.tile_pool(name="pz", bufs=4, space="PSUM") as pzp,
            tc.tile_pool(name="psm", bufs=1, space="PSUM") as psmp,
            tc.tile_pool(name="dram", bufs=1, space="DRAM") as dramp,
        ):
            # ---------- constants / small inputs ----------
            identt = constp.tile([128, 128], F32, tag="identt")
            nc.sync.dma_start(identt[:], ident[:])
            ones_row = constp.tile([1, 128], F32, tag="ones_row")
            nc.vector.memset(ones_row[:], 1.0)
            b1t = constp.tile([1, C], F32, tag="b1t")
            nc.sync.dma_start(b1t[:], b1[:])
            # einsum operands first: w0t shard (lhsT) + xT (rhs) gate
            # the critical path; w0n (weighted-sum, post-AR) goes last.
            # w0tt free layout: (s, dc) chunk of [128_d, 512_h]
            w0tt = bigp.tile([128, SL * NDC * H], F32, tag="w0tt")
            xtt = constp.tile([128, NDC * B], F32, tag="xtt")

            def load_w0t(s):
                return nc.sync.dma_start(
                    w0tt[:, s * NDC * H:(s + 1) * NDC * H].rearrange(
                        "p (c h) -> p c h", c=NDC),
                    w0t[s].rearrange("(c p) h -> p c h", p=128))

            # serialize the front loads: s0+xT get full DMA bandwidth so the
            # first matmul starts ~6us in; later chunks chase the einsum.
            i_prev = load_w0t(0)
            i_xtt = nc.sync.dma_start(
                xtt[:].rearrange("p (c b) -> p c b", c=NDC),
                xt[:].rearrange("(c p) b -> p c b", p=128))
            # b0 columns: [128_h, (hc, s)]
            b0cc = constp.tile([128, NHC * SL], F32, tag="b0cc")
            i_b0 = nc.sync.dma_start(
                b0cc[:].rearrange("p (c s) -> p c s", c=NHC),
                b0c[:].rearrange("(c p) s -> p c s", p=128))
            add_dep_helper(i_b0.ins, i_prev.ins, reason="front DMA ordering")
            for s in range(1, SL):
                i_s = load_w0t(s)
                add_dep_helper(i_s.ins, i_prev.ins,
                               reason="front DMA ordering")
                i_prev = i_s
            w1tt = constp.tile([128, NHC * C], F32, tag="w1tt")
            i_w1 = nc.sync.dma_start(
                w1tt[:].rearrange("p (c k) -> p c k", c=NHC),
                w1t[:].rearrange("(c p) k -> p c k", p=128))
            add_dep_helper(i_w1.ins, i_prev.ins, reason="front DMA ordering")
            w0nt = bigp.tile([128, SL * NHC * D], F32, tag="w0nt")
            if K_F32R:
                # fp32r matmul operands must be produced pre-rounded
                F32R = mybir.dt.float32r
                w0ttr = bigp.tile([128, SL * NDC * H], F32R, tag="w0ttr")
                xttr = constp.tile([128, NDC * B], F32R, tag="xttr")
                nc.vector.tensor_copy(xttr[:], xtt[:])
                for s in range(SL):
                    nc.vector.tensor_copy(
                        w0ttr[:, s * NDC * H:(s + 1) * NDC * H],
                        w0tt[:, s * NDC * H:(s + 1) * NDC * H])

            zts = {}
            xtl = []
            cc_in = dramp.tile([H, B], F32, tag="cc_in")
            cc_out = dramp.tile([H, B], F32, tag="cc_out", addr_space="Shared")
            for hc in range(NHC):
                mt = bigp.tile([128, B], F32, tag=f"xtl{hc}")
                xtl.append(mt)
            for s in range(SL):
                for hc in range(NHC):
                    pz = pzp.tile([128, B], F32, tag="pz")
                    for dc in range(NDC):
                        lsrc = w0ttr if K_F32R else w0tt
                        rsrc = xttr if K_F32R else xtt
                        lhs = lsrc[:, (s * NDC + dc) * H + hc * 128:
                                   (s * NDC + dc) * H + (hc + 1) * 128]
                        rhs = rsrc[:, dc * B:(dc + 1) * B]
                        nc.tensor.matmul(
                            pz[:], lhs, rhs,
                            start=(dc == 0), stop=(dc == NDC - 1))
                    zt = bigp.tile([128, B], F32, tag=f"z{s}_{hc}")
                    zts[(s, hc)] = zt
                    nc.scalar.activation(
                        zt[:], pz[:], AF.Identity,
                        bias=b0cc[:, hc * SL + s:hc * SL + s + 1])
                    # incremental running max per hc
                    if s == 1:
                        nc.vector.tensor_tensor(
                            out=xtl[hc][:], in0=zts[(0, hc)][:],
                            in1=zt[:], op=OP.max)
                    elif s > 1:
                        nc.vector.tensor_tensor(
                            out=xtl[hc][:], in0=xtl[hc][:],
                            in1=zt[:], op=OP.max)
                    if s == SL - 1:
                        nc.sync.dma_start(
                            cc_in[hc * 128:(hc + 1) * 128, :], xtl[hc][:])
            nc.gpsimd.collective_compute(
                "AllReduce", OP.max,
                replica_groups=[list(range(N_CORES))],
                ins=[cc_in[:].opt()], outs=[cc_out[:].opt()])
            # w0n (weighted-sum operand) loads during the AR dead time
            for s in range(SL):
                nc.sync.dma_start(
                    w0nt[:, s * NHC * D:(s + 1) * NHC * D].rearrange(
                        "p (c d) -> p c d", c=NHC),
                    w0n[s].rearrange("(c p) d -> p c d", p=128))
            gxt = []
            xrel = []
            for hc in range(NHC):
                gt = bigp.tile([128, B], F32, tag=f"g{hc}")
                gxt.append(gt)
                nc.sync.dma_start(gt[:], cc_out[hc * 128:(hc + 1) * 128, :])
                # relu early so the logits/softmax chain starts with eq
                xr = bigp.tile([128, B], F32, tag=f"xr{hc}")
                xrel.append(xr)
                nc.vector.tensor_scalar(
                    out=xr[:], in0=gt[:], scalar1=0.0, scalar2=None,
                    op0=OP.max)

            # ---------- upd share from sign(x) (independent of AR) -------
            # sum_b sign(x) = #pos - #neg, via tensor_scalar accumulators
            updcol = workp.tile([128, NDC], F32, tag="updcol", bufs=1)
            for dc in range(NDC):
                sx = workp.tile([128, B], F32, tag="sx", bufs=2)
                pos = workp.tile([128, 1], F32, tag="pos", bufs=2)
                neg = workp.tile([128, 1], F32, tag="neg", bufs=2)
                nc.vector.tensor_scalar(
                    out=sx[:], in0=xtt[:, dc * B:(dc + 1) * B],
                    scalar1=0.0, scalar2=0.0, op0=OP.is_gt, op1=OP.add,
                    accum_out=pos[:])
                nc.vector.tensor_scalar(
                    out=sx[:], in0=xtt[:, dc * B:(dc + 1) * B],
                    scalar1=0.0, scalar2=0.0, op0=OP.is_lt, op1=OP.add,
                    accum_out=neg[:])
                nc.vector.tensor_scalar(
                    out=updcol[:, dc:dc + 1], in0=pos[:],
                    scalar1=neg[:], scalar2=LIANN_LR / N_CORES,
                    op0=OP.subtract, op1=OP.mult)

            # ---------- fused eq+count, diag, weighted sum ----------
            countsT = workp.tile([128, NHC * SL], F32, tag="countsT", bufs=1)
            ptop = []
            for dc in range(NDC):
                pt = psmp.tile([128, H], F32, tag=f"ptop{dc}", bufs=1)
                ptop.append(pt)
            for hc in range(NHC):
                dgs = []
                for s in range(SL):
                    eqs = workp.tile([128, B], F32, tag="eqs", bufs=3)
                    if K_TTR:
                        nc.vector.tensor_tensor_reduce(
                            out=eqs[:], in0=zts[(s, hc)][:], in1=gxt[hc][:],
                            scale=1.0, scalar=0.0,
                            op0=OP.is_equal, op1=OP.add,
                            accum_out=countsT[:, hc * SL + s:hc * SL + s + 1])
                    else:
                        nc.vector.tensor_tensor(
                            out=eqs[:], in0=zts[(s, hc)][:], in1=gxt[hc][:],
                            op=OP.is_equal)
                        eq2 = workp.tile([128, B], F32, tag="eq2", bufs=3)
                        nc.scalar.activation(
                            eq2[:], eqs[:], AF.Identity,
                            accum_out=countsT[:, hc * SL + s:hc * SL + s + 1])
                    dg = workp.tile([128, 128], F32, tag=f"dg{s}", bufs=2)
                    dgs.append(dg)
                    nc.vector.tensor_scalar(
                        out=dg[:], in0=identt[:],
                        scalar1=countsT[:, hc * SL + s:hc * SL + s + 1],
                        scalar2=None, op0=OP.mult)
                for dc in range(NDC):
                    for s in range(SL):
                        nc.tensor.matmul(
                            ptop[dc][:, hc * 128:(hc + 1) * 128],
                            w0nt[:, (s * NHC + hc) * D + dc * 128:
                                 (s * NHC + hc) * D + (dc + 1) * 128],
                            dgs[s][:],
                            start=(s == 0), stop=(s == SL - 1))

            for dc in range(NDC):
                tk = workp.tile([128, H], F32, tag="tk", bufs=2)
                nc.vector.tensor_copy(tk[:], ptop[dc][:])
                nc.vector.tensor_scalar(
                    out=tk[:], in0=tk[:],
                    scalar1=updcol[:, dc:dc + 1], scalar2=None,
                    op0=OP.add)
                nc.sync.dma_start(topkt_out[dc * 128:(dc + 1) * 128, :], tk[:])

            # ---------- head: relu(xTop^T) @ W1^T + b1, log_softmax ------
            for bt in range(NBT):
                pl = pzp.tile([128, C], F32, tag="pz")
                for hc in range(NHC):
                    nc.tensor.matmul(
                        pl[:],
                        xrel[hc][:, bt * 128:(bt + 1) * 128],
                        w1tt[:, hc * C:(hc + 1) * C],
                        start=(hc == 0), stop=False)
                nc.tensor.matmul(
                    pl[:], ones_row[:], b1t[:], start=False, stop=True)
                # log-softmax over free dim (C)
                mx = workp.tile([128, 1], F32, tag="mx")
                nc.vector.reduce_max(mx[:], pl[:], axis=mybir.AxisListType.X)
                sh = workp.tile([128, C], F32, tag="sh")
                nc.vector.tensor_scalar(
                    out=sh[:], in0=pl[:], scalar1=mx[:], scalar2=None,
                    op0=OP.subtract)
                ex = workp.tile([128, C], F32, tag="ex")
                se = workp.tile([128, 1], F32, tag="se")
                nc.scalar.activation(ex[:], sh[:], AF.Exp, accum_out=se[:])
                ls = workp.tile([128, 1], F32, tag="ls")
                nc.scalar.activation(ls[:], se[:], AF.Ln)
                lp = workp.tile([128, C], F32, tag="lp")
                nc.vector.tensor_scalar(
                    out=lp[:], in0=sh[:], scalar1=ls[:], scalar2=None,
                    op0=OP.subtract)
                nc.sync.dma_start(logp_out[bt * 128:(bt + 1) * 128, :], lp[:])

    nc.finalize()
    return nc


def get_nc():
    global _CACHED_NC
    if _CACHED_NC is None:
        _CACHED_NC = build_nc()
    return _CACHED_NC


def make_in_maps(x, W0, b0, W1, b1):
    x = np.asarray(x, np.float32)
    W0 = np.asarray(W0, np.float32)
    b0 = np.asarray(b0, np.float32)
    W1 = np.asarray(W1, np.float32)
    b1 = np.asarray(b1, np.float32)

    xt = np.ascontiguousarray(x.T)                    # [D, B]
    w1t = np.ascontiguousarray(W1.T)                  # [H, C]
    b1r = np.ascontiguousarray(b1.reshape(1, C))
    ident = np.eye(128, dtype=np.float32)

    in_maps = []
    for c in range(N_CORES):
        sl = slice(c * SL, (c + 1) * SL)
        in_maps.append({
            "w0n": np.ascontiguousarray(W0[sl]),
            "w0t": np.ascontiguousarray(W0[sl].transpose(0, 2, 1)),
            "b0c": np.ascontiguousarray(b0[sl].T),    # [H, SL]
            "xt": xt,
            "w1t": w1t,
            "b1": b1r,
            "ident": ident,
        })
    return in_maps


def postprocess(results, y):
    logp = results[0]["logp"]
    topkt = sum(r["topkt"] for r in results)
    topk = np.ascontiguousarray(topkt.T) / np.float32(B)

    yi = np.asarray(y).astype(np.int64)
    loss = np.float32(-np.mean(logp[np.arange(B), yi]))
    acc = np.float32(np.mean((np.argmax(logp, axis=-1) == yi)))
    return loss, acc, logp, topk


def kernel(x, y, W0, b0, W1, b1):
    nc = get_nc()
    in_maps = make_in_maps(x, W0, b0, W1, b1)
    res = bass_utils.run_bass_kernel_spmd(
        nc, in_maps, core_ids=list(range(N_CORES)))
    return postprocess(res.results, y)


# revision 26
# speedup vs baseline: 1.1113x; 1.0015x over previous
"""Trainium2 Bass kernel for nn_LIANNmodel (moe_routing).

Model (B=512, S=32 experts, H=512, D=256, C=128):
    z[b,s,h]   = sum_d x[b,d] * W0[s,h,d] + b0[s,h]
    xTop       = max_s z ; xIndex = argmax_s z
    topK[h,d]  = mean_b( W0[xIndex[b,h], h, d] + LR*sign(x[b,d]) )
    logits     = relu(xTop) @ W1.T + b1 ; logp = log_softmax(logits)
    loss, acc  from logp vs labels y

Sharding: expert-sharded over S across 8 cores (4 experts/core).  Each core
computes z^T (h on partitions, b on free) for its experts over the full
batch, reduces a local max, and an AllReduce(max) of [H,B] produces the
global xTop^T on every core.  Routing counts are recovered locally with a
fused is_equal+reduce against xTop^T (no all-to-all needed), and the
gather-mean  sum_b W0[xIndex[b,h],h,:]  is computed on the tensor engine as
sum_s  W0[s]-native-block^T @ diag(count[s,.])  accumulated in PSUM.  The
classifier head is computed redundantly on every core (xTop^T is already
the needed lhsT).  The host only shards inputs, sums the 8 partial topK
outputs, and computes the two scalar reductions (loss/accuracy) from the
device-produced logp.

Layout rules learned from hardware: only contiguous DMA (strided/transposed
DRAM reads run at ~5 GB/s vs 165 GB/s); all transposes happen on the host
(x.T, W1.T, b0.T - free) or on the PE via identity matmuls (W0 blocks).
The z^T layout makes b0 a per-partition column folded into the PSUM->SBUF
copy on the scalar engine, and makes counts per-partition columns exactly
as the diag build needs - no bias matmuls, no count/xTop transposes.
"""

import sys
import os
import numpy as np

for _p in ("/opt/trn_rl_repo", "/root/.axon_site/_ro/trn_rl_repo"):
    if os.path.isdir(_p) and _p not in sys.path:
        sys.path.append(_p)

import concourse.bacc as bacc
from concourse.tile import add_dep_helper
import concourse.mybir as mybir
import concourse.tile as tile
from concourse import bass_utils

F32 = mybir.dt.float32
AF = mybir.ActivationFunctionType
OP = mybir.AluOpType

B, S, H, D, C = 512, 32, 512, 256, 128
N_CORES = 8
SL = S // N_CORES          # experts per core (shard length)
NBT = B // 128             # batch tiles of 128
NHC = H // 128             # h chunks of 128
NDC = D // 128             # d chunks of 128
LIANN_LR = 0.001

_CACHED_NC = None
# tensor_tensor_reduce crashes at runtime on HW (sim-only ok)
K_TTR = os.environ.get("K_TTR", "0") == "1"
K_DMA3D = os.environ.get("K_DMA3D", "1") == "1"
# fp32r: reduced-precision multiply, 4x matmul stream rate at N>=256
K_F32R = os.environ.get("K_F32R", "0") == "1"


def build_nc():
    nc = bacc.Bacc("TRN2", target_bir_lowering=False, debug=False,
                   num_devices=N_CORES)

    # ---- kernel I/O (per-core shapes; contents differ per core) ----
    w0n = nc.dram_tensor("w0n", [SL, H, D], F32, kind="ExternalInput")
    w0t = nc.dram_tensor("w0t", [SL, D, H], F32, kind="ExternalInput")
    b0c = nc.dram_tensor("b0c", [H, SL], F32, kind="ExternalInput")  # b0.T
    xt = nc.dram_tensor("xt", [D, B], F32, kind="ExternalInput")     # x.T
    w1t = nc.dram_tensor("w1t", [H, C], F32, kind="ExternalInput")   # W1.T
    b1 = nc.dram_tensor("b1", [1, C], F32, kind="ExternalInput")
    ident = nc.dram_tensor("ident", [128, 128], F32, kind="ExternalInput")

    logp_out = nc.dram_tensor("logp", [B, C], F32, kind="ExternalOutput")
    # transposed partial: sum_{s in shard} count[s,h]*W0[s,h,d]  (+ upd share)
    topkt_out = nc.dram_tensor("topkt", [D, H], F32, kind="ExternalOutput")

    with tile.TileContext(nc) as tc:
        with (
            tc.tile_pool(name="const", bufs=1) as constp,
            tc.tile_pool(name="big", bufs=1) as bigp,
            tc.tile_pool(name="work", bufs=3) as workp,
            tc.tile_pool(name="pz", bufs=4, space="PSUM") as pzp,
            tc.tile_pool(name="psm", bufs=1, space="PSUM") as psmp,
            tc.tile_pool(name="dram", bufs=1, space="DRAM") as dramp,
        ):
            # ---------- constants / small inputs ----------
            identt = constp.tile([128, 128], F32, tag="identt")
            nc.sync.dma_start(identt[:], ident[:])
            ones_row = constp.tile([1, 128], F32, tag="ones_row")
            nc.vector.memset(ones_row[:], 1.0)
            b1t = constp.tile([1, C], F32, tag="b1t")
            nc.sync.dma_start(b1t[:], b1[:])
            # einsum operands first: w0t shard (lhsT) + xT (rhs) gate
            # the critical path; w0n (weighted-sum, post-AR) goes last.
            # w0tt free layout: (s, dc) chunk of [128_d, 512_h]
            w0tt = bigp.tile([128, SL * NDC * H], F32, tag="w0tt")
            xtt = constp.tile([128, NDC * B], F32, tag="xtt")

            def load_w0t(s):
                return nc.sync.dma_start(
                    w0tt[:, s * NDC * H:(s + 1) * NDC * H].rearrange(
                        "p (c h) -> p c h", c=NDC),
                    w0t[s].rearrange("(c p) h -> p c h", p=128))

            # serialize the front loads in consumption order: the dc0 halves
            # of w0t[s0] and xT land first (512KB) so the first matmul can
            # issue early; the dc1 halves and later chunks chase the einsum.
            i_a = nc.sync.dma_start(w0tt[:, 0:H], w0t[0, 0:128, :])
            i_b = nc.sync.dma_start(xtt[:, 0:B], xt[0:128, :])
            i_c = nc.sync.dma_start(w0tt[:, H:2 * H], w0t[0, 128:256, :])
            i_d = nc.sync.dma_start(xtt[:, B:2 * B], xt[128:256, :])
            add_dep_helper(i_c.ins, i_a.ins, reason="front DMA ordering")
            add_dep_helper(i_d.ins, i_b.ins, reason="front DMA ordering")
            i_prev = i_c
            # b0 columns: [128_h, (hc, s)]
            b0cc = constp.tile([128, NHC * SL], F32, tag="b0cc")
            i_b0 = nc.sync.dma_start(
                b0cc[:].rearrange("p (c s) -> p c s", c=NHC),
                b0c[:].rearrange("(c p) s -> p c s", p=128))
            add_dep_helper(i_b0.ins, i_prev.ins, reason="front DMA ordering")
            for s in range(1, SL):
                i_s = load_w0t(s)
                add_dep_helper(i_s.ins, i_prev.ins,
                               reason="front DMA ordering")
                i_prev = i_s
            w1tt = constp.tile([128, NHC * C], F32, tag="w1tt")
            i_w1 = nc.sync.dma_start(
                w1tt[:].rearrange("p (c k) -> p c k", c=NHC),
                w1t[:].rearrange("(c p) k -> p c k", p=128))
            add_dep_helper(i_w1.ins, i_prev.ins, reason="front DMA ordering")
            w0nt = bigp.tile([128, SL * NHC * D], F32, tag="w0nt")
            if K_F32R:
                # fp32r matmul operands must be produced pre-rounded
                F32R = mybir.dt.float32r
                w0ttr = bigp.tile([128, SL * NDC * H], F32R, tag="w0ttr")
                xttr = constp.tile([128, NDC * B], F32R, tag="xttr")
                nc.vector.tensor_copy(xttr[:], xtt[:])
                for s in range(SL):
                    nc.vector.tensor_copy(
                        w0ttr[:, s * NDC * H:(s + 1) * NDC * H],
                        w0tt[:, s * NDC * H:(s + 1) * NDC * H])

            zts = {}
            xtl = []
            cc_in = dramp.tile([H, B], F32, tag="cc_in")
            cc_out = dramp.tile([H, B], F32, tag="cc_out", addr_space="Shared")
            for hc in range(NHC):
                mt = bigp.tile([128, B], F32, tag=f"xtl{hc}")
                xtl.append(mt)
            for s in range(SL):
                for hc in range(NHC):
                    pz = pzp.tile([128, B], F32, tag="pz")
                    for dc in range(NDC):
                        lsrc = w0ttr if K_F32R else w0tt
                        rsrc = xttr if K_F32R else xtt
                        lhs = lsrc[:, (s * NDC + dc) * H + hc * 128:
                                   (s * NDC + dc) * H + (hc + 1) * 128]
                        rhs = rsrc[:, dc * B:(dc + 1) * B]
                        nc.tensor.matmul(
                            pz[:], lhs, rhs,
                            start=(dc == 0), stop=(dc == NDC - 1))
                    zt = bigp.tile([128, B], F32, tag=f"z{s}_{hc}")
                    zts[(s, hc)] = zt
                    nc.scalar.activation(
                        zt[:], pz[:], AF.Identity,
                        bias=b0cc[:, hc * SL + s:hc * SL + s + 1])
                    # incremental running max per hc
                    if s == 1:
                        nc.vector.tensor_tensor(
                            out=xtl[hc][:], in0=zts[(0, hc)][:],
                            in1=zt[:], op=OP.max)
                    elif s > 1:
                        nc.vector.tensor_tensor(
                            out=xtl[hc][:], in0=xtl[hc][:],
                            in1=zt[:], op=OP.max)
                    if s == SL - 1:
                        nc.sync.dma_start(
                            cc_in[hc * 128:(hc + 1) * 128, :], xtl[hc][:])
            nc.gpsimd.collective_compute(
                "AllReduce", OP.max,
                replica_groups=[list(range(N_CORES))],
                ins=[cc_in[:].opt()], outs=[cc_out[:].opt()])
            # w0n (weighted-sum operand) loads during the AR dead time
            for s in range(SL):
                nc.sync.dma_start(
                    w0nt[:, s * NHC * D:(s + 1) * NHC * D].rearrange(
                        "p (c d) -> p c d", c=NHC),
                    w0n[s].rearrange("(c p) d -> p c d", p=128))
            gxt = []
            xrel = []
            for hc in range(NHC):
                gt = bigp.tile([128, B], F32, tag=f"g{hc}")
                gxt.append(gt)
                nc.sync.dma_start(gt[:], cc_out[hc * 128:(hc + 1) * 128, :])
                # relu early so the logits/softmax chain starts with eq
                xr = bigp.tile([128, B], F32, tag=f"xr{hc}")
                xrel.append(xr)
                nc.vector.tensor_scalar(
                    out=xr[:], in0=gt[:], scalar1=0.0, scalar2=None,
                    op0=OP.max)

            # ---------- upd share from sign(x) (independent of AR) -------
            # sum_b sign(x) = #pos - #neg, via tensor_scalar accumulators
            updcol = workp.tile([128, NDC], F32, tag="updcol", bufs=1)
            for dc in range(NDC):
                sx = workp.tile([128, B], F32, tag="sx", bufs=2)
                pos = workp.tile([128, 1], F32, tag="pos", bufs=2)
                neg = workp.tile([128, 1], F32, tag="neg", bufs=2)
                nc.vector.tensor_scalar(
                    out=sx[:], in0=xtt[:, dc * B:(dc + 1) * B],
                    scalar1=0.0, scalar2=0.0, op0=OP.is_gt, op1=OP.add,
                    accum_out=pos[:])
                nc.vector.tensor_scalar(
                    out=sx[:], in0=xtt[:, dc * B:(dc + 1) * B],
                    scalar1=0.0, scalar2=0.0, op0=OP.is_lt, op1=OP.add,
                    accum_out=neg[:])
                nc.vector.tensor_scalar(
                    out=updcol[:, dc:dc + 1], in0=pos[:],
                    scalar1=neg[:], scalar2=LIANN_LR / N_CORES,
                    op0=OP.subtract, op1=OP.mult)

            # ---------- fused eq+count, diag, weighted sum ----------
            countsT = workp.tile([128, NHC * SL], F32, tag="countsT", bufs=1)
            ptop = []
            for dc in range(NDC):
                pt = psmp.tile([128, H], F32, tag=f"ptop{dc}", bufs=1)
                ptop.append(pt)
            for hc in range(NHC):
                dgs = []
                for s in range(SL):
                    eqs = workp.tile([128, B], F32, tag="eqs", bufs=3)
                    if K_TTR:
                        nc.vector.tensor_tensor_reduce(
                            out=eqs[:], in0=zts[(s, hc)][:], in1=gxt[hc][:],
                            scale=1.0, scalar=0.0,
                            op0=OP.is_equal, op1=OP.add,
                            accum_out=countsT[:, hc * SL + s:hc * SL + s + 1])
                    else:
                        nc.vector.tensor_tensor(
                            out=eqs[:], in0=zts[(s, hc)][:], in1=gxt[hc][:],
                            op=OP.is_equal)
                        eq2 = workp.tile([128, B], F32, tag="eq2", bufs=3)
                        nc.scalar.activation(
                            eq2[:], eqs[:], AF.Identity,
                            accum_out=countsT[:, hc * SL + s:hc * SL + s + 1])
                    dg = workp.tile([128, 128], F32, tag=f"dg{s}", bufs=2)
                    dgs.append(dg)
                    nc.vector.tensor_scalar(
                        out=dg[:], in0=identt[:],
                        scalar1=countsT[:, hc * SL + s:hc * SL + s + 1],
                        scalar2=None, op0=OP.mult)
                for dc in range(NDC):
                    for s in range(SL):
                        nc.tensor.matmul(
                            ptop[dc][:, hc * 128:(hc + 1) * 128],
                            w0nt[:, (s * NHC + hc) * D + dc * 128:
                                 (s * NHC + hc) * D + (dc + 1) * 128],
                            dgs[s][:],
                            start=(s == 0), stop=(s == SL - 1))

            for dc in range(NDC):
                tk = workp.tile([128, H], F32, tag="tk", bufs=2)
                nc.vector.tensor_copy(tk[:], ptop[dc][:])
                nc.vector.tensor_scalar(
                    out=tk[:], in0=tk[:],
                    scalar1=updcol[:, dc:dc + 1], scalar2=None,
                    op0=OP.add)
                nc.sync.dma_start(topkt_out[dc * 128:(dc + 1) * 128, :], tk[:])

            # ---------- head: relu(xTop^T) @ W1^T + b1, log_softmax ------
            for bt in range(NBT):
                pl = pzp.tile([128, C], F32, tag="pz")
                for hc in range(NHC):
                    nc.tensor.matmul(
                        pl[:],
                        xrel[hc][:, bt * 128:(bt + 1) * 128],
                        w1tt[:, hc * C:(hc + 1) * C],
                        start=(hc == 0), stop=False)
                nc.tensor.matmul(
                    pl[:], ones_row[:], b1t[:], start=False, stop=True)
                # log-softmax over free dim (C)
                mx = workp.tile([128, 1], F32, tag="mx")
                nc.vector.reduce_max(mx[:], pl[:], axis=mybir.AxisListType.X)
                sh = workp.tile([128, C], F32, tag="sh")
                nc.vector.tensor_scalar(
                    out=sh[:], in0=pl[:], scalar1=mx[:], scalar2=None,
                    op0=OP.subtract)
                ex = workp.tile([128, C], F32, tag="ex")
                se = workp.tile([128, 1], F32, tag="se")
                nc.scalar.activation(ex[:], sh[:], AF.Exp, accum_out=se[:])
                ls = workp.tile([128, 1], F32, tag="ls")
                nc.scalar.activation(ls[:], se[:], AF.Ln)
                lp = workp.tile([128, C], F32, tag="lp")
                nc.vector.tensor_scalar(
                    out=lp[:], in0=sh[:], scalar1=ls[:], scalar2=None,
                    op0=OP.subtract)
                nc.sync.dma_start(logp_out[bt * 128:(bt + 1) * 128, :], lp[:])

    nc.finalize()
    return nc


def get_nc():
    global _CACHED_NC
    if _CACHED_NC is None:
        _CACHED_NC = build_nc()
    return _CACHED_NC


def make_in_maps(x, W0, b0, W1, b1):
    x = np.asarray(x, np.float32)
    W0 = np.asarray(W0, np.float32)
    b0 = np.asarray(b0, np.float32)
    W1 = np.asarray(W1, np.float32)
    b1 = np.asarray(b1, np.float32)

    xt = np.ascontiguousarray(x.T)                    # [D, B]
    w1t = np.ascontiguousarray(W1.T)                  # [H, C]
    b1r = np.ascontiguousarray(b1.reshape(1, C))
    ident = np.eye(128, dtype=np.float32)

    in_maps = []
    for c in range(N_CORES):
        sl = slice(c * SL, (c + 1) * SL)
        in_maps.append({
            "w0n": np.ascontiguousarray(W0[sl]),
            "w0t": np.ascontiguousarray(W0[sl].transpose(0, 2, 1)),
            "b0c": np.ascontiguousarray(b0[sl].T),    # [H, SL]
            "xt": xt,
            "w1t": w1t,
            "b1": b1r,
            "ident": ident,
        })
    return in_maps


def postprocess(results, y):
    logp = results[0]["logp"]
    topkt = sum(r["topkt"] for r in results)
    topk = np.ascontiguousarray(topkt.T) / np.float32(B)

    yi = np.asarray(y).astype(np.int64)
    loss = np.float32(-np.mean(logp[np.arange(B), yi]))
    acc = np.float32(np.mean((np.argmax(logp, axis=-1) == yi)))
    return loss, acc, logp, topk


def kernel(x, y, W0, b0, W1, b1):
    nc = get_nc()
    in_maps = make_in_maps(x, W0, b0, W1, b1)
    res = bass_utils.run_bass_kernel_spmd(
        nc, in_maps, core_ids=list(range(N_CORES)))
    return postprocess(res.results, y)
